# revision 1
# baseline (speedup 1.0000x reference)
"""LocalSelfAttention (window=7) Trainium2 Bass kernel.

Full inputs in, full output out. Sharding: 8 cores = batch(4) x seq-half(2),
each core handles 1024 tokens with a 3-token zero-padded halo on xs.

Math notes (exact rewrites of the reference):
- reference projects zero-PADDED xs patches, so out-of-range taps have
  k = b_ks, v = b_vs. Softmax over taps is invariant to the per-(t,h)
  constant q . b_ks, so the K bias drops entirely (padded taps then score 0,
  matching zero-padded halo @ w_ks with no bias).
- softmax weights sum to 1, so the V bias contributes exactly b_vs to o;
  it is folded on the host into b' = b_vs @ w_fc + b_fc.

Pipeline per core (feature-major activations, transposed on the HOST):
- QT/KT feature-major via matmul(lhsT=weight tile, rhs=xT), V token-major
  via matmul(lhsT=xsT slice, rhs=weight tile).
- attention in 9 chunks of 122 tokens (last chunk overlaps, recomputing a
  74-token stripe, so every window is exactly 128 wide and each PV matmul
  needs a single V partition-tile), processed in head PAIRS:
  scores for both heads land in one PSUM tile (122, 256), band-masked
  softmax with wide DVE/ACT ops, PE-transpose of the prob tile, one PV
  matmul per head accumulating both heads into one (128, 122) PSUM tile.
- FC + rank-1 bias matmul + residual + layernorm (split copy/add/reduce on
  DVE: a fused PSUM-source tensor_tensor_reduce crashes the exec unit).
"""

import sys

for _p in ("/opt/trn_rl_repo",):
    if _p not in sys.path:
        sys.path.insert(0, _p)

import numpy as np
import ml_dtypes

BF16 = ml_dtypes.bfloat16

H, DK, DV, D = 16, 64, 64, 1024
NEI = 3
TEMP = 8.0
EPS = 1e-5
B, S = 4, 2048
NCORES = 8
T = (B * S) // NCORES          # 1024 tokens per core
TH = T + 2 * NEI               # 1030 halo tokens
P = 128
NT = T // P                    # 8 fc-phase token chunks
ND = D // P                    # 8 feature chunks
CL = 96                        # attention chunk length (PE M must be x32)
CST = [96 * i for i in range(10)] + [928]          # chunk starts
TH2 = 1056                     # padded halo width (window reads up to 1056)
NEG = -30000.0

_CACHE = {}


def _build_program(apply_affine: bool):
    import concourse.bacc as bacc
    import concourse.tile as tile
    from concourse import mybir
    from contextlib import ExitStack

    f32 = mybir.dt.float32
    bf16 = mybir.dt.bfloat16
    Alu = mybir.AluOpType
    Act = mybir.ActivationFunctionType

    nc = bacc.Bacc(
        "TRN2", target_bir_lowering=False, debug=False, enable_asserts=False
    )

    def din(name, shape, dt_):
        return nc.dram_tensor(name, shape, dt_, kind="ExternalInput").ap()

    xq32 = din("xq32", (T, D), f32)      # residual (token-major, f32)
    xqT = din("xqT", (D, T), bf16)       # x^T (host-transposed)
    xsT = din("xsT", (D, TH), bf16)      # xs^T with halo (host-transposed)
    wq = din("wq", (D, D), bf16)
    wk = din("wk", (D, D), bf16)
    wv = din("wv", (D, D), bf16)
    wf = din("wf", (D, D), bf16)
    bq = din("bq", (P, ND), f32)         # b_qs laid out [p, ec]
    bpr = din("bpr", (1, D), bf16)       # b_vs @ w_fc + b_fc
    msk = din("msk", (CL, 2 * P), f32)   # band mask 0 / NEG, two head slots
    idn = din("idn", (P, P), bf16)       # identity for PE transpose
    ones = din("ones", (1, P), bf16)     # rank-1 bias helper
    if apply_affine:
        lng = din("lng", (1, D), f32)
        lnb = din("lnb", (1, D), f32)
    yo = nc.dram_tensor("yo", (T, D), f32, kind="ExternalOutput").ap()

    with tile.TileContext(nc) as tc, ExitStack() as ctx:
        consts = ctx.enter_context(tc.tile_pool(name="consts", bufs=1))
        big = ctx.enter_context(tc.tile_pool(name="big", bufs=1))
        wpool = ctx.enter_context(tc.tile_pool(name="wpool", bufs=2))
        xrpool = ctx.enter_context(tc.tile_pool(name="xrpool", bufs=3))
        work = ctx.enter_context(tc.tile_pool(name="work", bufs=3))
        lnpool = ctx.enter_context(tc.tile_pool(name="lnpool", bufs=2))
        small = ctx.enter_context(tc.tile_pool(name="small", bufs=4))
        psA = ctx.enter_context(tc.tile_pool(name="psA", bufs=3, space="PSUM"))
        psS = ctx.enter_context(tc.tile_pool(name="psS", bufs=1, space="PSUM"))
        psT = ctx.enter_context(tc.tile_pool(name="psT", bufs=2, space="PSUM"))
        psO = ctx.enter_context(tc.tile_pool(name="psO", bufs=1, space="PSUM"))

        # ---- constants ----
        msk_sb = consts.tile([CL, 2 * P], f32, tag="msk")
        nc.sync.dma_start(out=msk_sb, in_=msk)
        idn_sb = consts.tile([P, P], bf16, tag="idn")
        nc.sync.dma_start(out=idn_sb, in_=idn)
        bq_sb = consts.tile([P, ND], f32, tag="bq")
        nc.sync.dma_start(out=bq_sb, in_=bq)
        bpr_sb = consts.tile([1, D], bf16, tag="bpr")
        nc.sync.dma_start(out=bpr_sb, in_=bpr)
        ones_sb = consts.tile([1, P], bf16, tag="ones")
        nc.sync.dma_start(out=ones_sb, in_=ones)
        eps_sb = consts.tile([P, 1], f32, tag="eps")
        nc.vector.memset(eps_sb, EPS)
        if apply_affine:
            import concourse.bass as bass

            g_bc = consts.tile([P, D], f32, tag="g_bc")
            b_bc = consts.tile([P, D], f32, tag="b_bc")
            nc.sync.dma_start(
                out=g_bc,
                in_=bass.AP(tensor=lng.tensor, offset=lng.offset,
                            ap=[[0, P]] + list(lng.ap[1:])),
            )
            nc.sync.dma_start(
                out=b_bc,
                in_=bass.AP(tensor=lnb.tensor, offset=lnb.offset,
                            ap=[[0, P]] + list(lnb.ap[1:])),
            )

        # ---- transposed activations: straight row-slice loads ----
        xT_t = []
        xsT_t = []
        for dc in range(ND):
            t1 = big.tile([P, T], bf16, tag=f"xT{dc}", name=f"xT{dc}")
            nc.sync.dma_start(out=t1, in_=xqT[dc * P:(dc + 1) * P, :])
            xT_t.append(t1)
        for dc in range(ND):
            t2 = big.tile([P, TH2], bf16, tag=f"xsT{dc}", name=f"xsT{dc}")
            nc.sync.dma_start(out=t2[:, 0:TH], in_=xsT[dc * P:(dc + 1) * P, :])
            nc.vector.memset(t2[:, TH:TH2], 0.0)
            xsT_t.append(t2)

        def load_w(wap, tagp):
            tiles = []
            for dc in range(ND):
                wt = wpool.tile([P, D], bf16, tag=f"w{dc}", name=f"w_{tagp}{dc}")
                nc.sync.dma_start(out=wt, in_=wap[dc * P:(dc + 1) * P, :])
                tiles.append(wt)
            return tiles

        # ---- QT projection: (e, t) feature-major, bias via ACT evict ----
        wq_t = load_w(wq, "q")
        QT = [big.tile([P, T], bf16, tag=f"QT{ec}", name=f"QT{ec}")
              for ec in range(ND)]
        for ec in range(ND):
            psa = psA.tile([P, 512], f32, tag="psA", name="ps_qa")
            psb = psA.tile([P, 512], f32, tag="psA", name="ps_qb")
            for dc in range(ND):
                lt = wq_t[dc][:, ec * P:(ec + 1) * P]
                nc.tensor.matmul(psa, lhsT=lt, rhs=xT_t[dc][:, 0:512],
                                 start=(dc == 0), stop=(dc == ND - 1))
                nc.tensor.matmul(psb, lhsT=lt, rhs=xT_t[dc][:, 512:1024],
                                 start=(dc == 0), stop=(dc == ND - 1))
            nc.scalar.activation(out=QT[ec][:, 0:512], in_=psa,
                                 func=Act.Identity,
                                 bias=bq_sb[:, ec:ec + 1], scale=1.0)
            nc.scalar.activation(out=QT[ec][:, 512:1024], in_=psb,
                                 func=Act.Identity,
                                 bias=bq_sb[:, ec:ec + 1], scale=1.0)

        # ---- KT projection: (e, t_halo) feature-major, no bias ----
        wk_t = load_w(wk, "k")
        KT = [big.tile([P, TH2], bf16, tag=f"KT{ec}", name=f"KT{ec}")
              for ec in range(ND)]
        for ec in range(ND):
            psa = psA.tile([P, 512], f32, tag="psA", name="ps_ka")
            psb = psA.tile([P, 512], f32, tag="psA", name="ps_kb")
            for dc in range(ND):
                lt = wk_t[dc][:, ec * P:(ec + 1) * P]
                nc.tensor.matmul(psa, lhsT=lt, rhs=xsT_t[dc][:, 0:512],
                                 start=(dc == 0), stop=(dc == ND - 1))
                nc.tensor.matmul(psb, lhsT=lt, rhs=xsT_t[dc][:, 512:1024],
                                 start=(dc == 0), stop=(dc == ND - 1))
            nc.scalar.activation(out=KT[ec][:, 0:512], in_=psa, func=Act.Copy)
            nc.scalar.activation(out=KT[ec][:, 512:1024], in_=psb,
                                 func=Act.Copy)
        for ec in range(ND):  # halo tail (incl zero padding)
            pst = psA.tile([P, TH2 - T], f32, tag="psA", name="ps_kt")
            for dc in range(ND):
                nc.tensor.matmul(pst,
                                 lhsT=wk_t[dc][:, ec * P:(ec + 1) * P],
                                 rhs=xsT_t[dc][:, T:TH2],
                                 start=(dc == 0), stop=(dc == ND - 1))
            nc.vector.tensor_copy(KT[ec][:, T:TH2], pst)

        # ---- V projection: token-major (halo-rows, e); 9 chunk tiles ----
        wv_t = load_w(wv, "v")
        V = []
        for ci, s in enumerate(CST):
            vt = big.tile([P, D], bf16, tag=f"V{ci}", name=f"V{ci}")
            psa = psA.tile([P, 512], f32, tag="psA", name="ps_va")
            psb = psA.tile([P, 512], f32, tag="psA", name="ps_vb")
            for dc in range(ND):
                lt = xsT_t[dc][:, s:s + P]
                nc.tensor.matmul(psa, lhsT=lt, rhs=wv_t[dc][:, 0:512],
                                 start=(dc == 0), stop=(dc == ND - 1))
                nc.tensor.matmul(psb, lhsT=lt, rhs=wv_t[dc][:, 512:1024],
                                 start=(dc == 0), stop=(dc == ND - 1))
            nc.scalar.activation(out=vt[:, 0:512], in_=psa, func=Act.Copy)
            nc.scalar.activation(out=vt[:, 512:1024], in_=psb, func=Act.Copy)
            V.append(vt)

        # prefetch FC weights during attention
        wf_t = load_w(wf, "f")

        # ---- windowed attention: chunks of 96, head pairs ----
        OT = [big.tile([P, T], bf16, tag=f"OT{ec}", name=f"OT{ec}")
              for ec in range(ND)]
        for ci, s in enumerate(CST):
            for ec in range(ND):  # head pair (2*ec, 2*ec+1)
                # scores for the two heads go to the two BANKS of one psum
                # tile (two matmuls into one bank crash the exec unit, and
                # matmul M must be a multiple of 32)
                s2 = psS.tile([CL, 1024], f32, tag="psS", name="s2")
                nc.tensor.matmul(
                    s2[:, 0:P],
                    lhsT=QT[ec][0:64, s:s + CL],
                    rhs=KT[ec][0:64, s:s + P],
                    start=True, stop=True,
                )
                nc.tensor.matmul(
                    s2[:, 512:512 + P],
                    lhsT=QT[ec][64:128, s:s + CL],
                    rhs=KT[ec][64:128, s:s + P],
                    start=True, stop=True,
                )
                sv = s2.rearrange("p (b c) -> p b c", b=2)[:, :, 0:P]
                pm2 = work.tile([CL, 2 * P], f32, tag="pm2", name="pm2")
                nc.vector.scalar_tensor_tensor(
                    out=pm2.rearrange("p (b c) -> p b c", b=2),
                    in0=sv, scalar=1.0 / TEMP,
                    in1=msk_sb.rearrange("p (b c) -> p b c", b=2),
                    op0=Alu.mult, op1=Alu.add,
                )
                pe2 = work.tile([CL, 2 * P], f32, tag="pe2", name="pe2")
                nc.scalar.activation(out=pe2, in_=pm2, func=Act.Exp)
                rs2 = small.tile([CL, 2], f32, tag="rs2", name="rs2")
                nc.vector.tensor_reduce(
                    out=rs2,
                    in_=pe2.rearrange("a (h w) -> a h w", h=2),
                    axis=mybir.AxisListType.X, op=Alu.add,
                )
                rsr2 = small.tile([CL, 2], f32, tag="rsr2", name="rsr2")
                nc.vector.reciprocal(rsr2, rs2)
                pn2 = work.tile([CL, 2 * P], bf16, tag="pn2", name="pn2")
                nc.vector.tensor_tensor(
                    pn2.rearrange("a (h w) -> a h w", h=2),
                    pe2.rearrange("a (h w) -> a h w", h=2),
                    rsr2[:, :, None].to_broadcast((CL, 2, P)),
                    Alu.mult,
                )
                pt_ps = psT.tile([P, 2 * P], bf16, tag="psT", name="pt_ps")
                nc.tensor.transpose(pt_ps[:, 0:CL], pn2[:, 0:P],
                                    idn_sb[0:CL, 0:CL])
                nc.tensor.transpose(pt_ps[:, P:P + CL], pn2[:, P:2 * P],
                                    idn_sb[0:CL, 0:CL])
                pt_sb = work.tile([P, 2 * P], bf16, tag="ptsb", name="pt_sb")
                nc.vector.tensor_copy(pt_sb[:, 0:CL], pt_ps[:, 0:CL])
                nc.scalar.activation(out=pt_sb[:, P:P + CL],
                                     in_=pt_ps[:, P:P + CL], func=Act.Copy)
                ot2 = psO.tile([P, CL], f32, tag="psO", name="ot2")
                nc.tensor.matmul(
                    ot2[0:64, :],
                    lhsT=V[ci][:, ec * P:ec * P + 64],
                    rhs=pt_sb[:, 0:CL], start=True, stop=True,
                )
                nc.tensor.matmul(
                    ot2[64:128, :],
                    lhsT=V[ci][:, ec * P + 64:(ec + 1) * P],
                    rhs=pt_sb[:, P:P + CL], start=True, stop=True,
                )
                nc.scalar.activation(out=OT[ec][:, s:s + CL], in_=ot2,
                                     func=Act.Copy)

        # ---- FC + rank-1 bias + residual + layernorm ----
        for c in range(NT):
            cs = slice(c * P, (c + 1) * P)
            xr = xrpool.tile([P, D], f32, tag="xr", name="xr")
            nc.sync.dma_start(out=xr, in_=xq32[cs, :])
            y_sb = lnpool.tile([P, D], f32, tag="ysb", name="y_sb")
            psa = psA.tile([P, 512], f32, tag="psA", name="ps_fa")
            psb = psA.tile([P, 512], f32, tag="psA", name="ps_fb")
            for ec in range(ND):
                lt = OT[ec][:, cs]
                nc.tensor.matmul(psa, lhsT=lt, rhs=wf_t[ec][:, 0:512],
                                 start=(ec == 0), stop=False)
                nc.tensor.matmul(psb, lhsT=lt, rhs=wf_t[ec][:, 512:1024],
                                 start=(ec == 0), stop=False)
            nc.tensor.matmul(psa, lhsT=ones_sb, rhs=bpr_sb[:, 0:512],
                             start=False, stop=True)
            nc.tensor.matmul(psb, lhsT=ones_sb, rhs=bpr_sb[:, 512:1024],
                             start=False, stop=True)
            ysum = None
            for dcol, ps in ((0, psa), (1, psb)):
                ds_ = slice(dcol * 512, (dcol + 1) * 512)
                # NOTE: fused PSUM-source tensor_tensor_reduce crashes the
                # exec unit on HW; split copy + sbuf add + reduce.
                nc.vector.tensor_copy(y_sb[:, ds_], ps)
                nc.vector.tensor_add(y_sb[:, ds_], y_sb[:, ds_], xr[:, ds_])
                new_sum = small.tile([P, 1], f32, tag=f"ysum{dcol}",
                                     name="ysum")
                nc.vector.tensor_reduce(
                    out=new_sum, in_=y_sb[:, ds_],
                    axis=mybir.AxisListType.X, op=Alu.add,
                )
                if ysum is not None:
                    nsum2 = small.tile([P, 1], f32, tag="nsum2", name="nsum2")
                    nc.vector.tensor_add(nsum2, new_sum, ysum)
                    new_sum = nsum2
                ysum = new_sum
            sqs = []
            for dcol in range(2):
                ds_ = slice(dcol * 512, (dcol + 1) * 512)
                ysq = lnpool.tile([P, 512], f32, tag="ysq", name="ysq")
                sq = small.tile([P, 1], f32, tag=f"sq{dcol}", name="sq")
                nc.scalar.activation(out=ysq, in_=y_sb[:, ds_],
                                     func=Act.Square, accum_out=sq)
                sqs.append(sq)
            ssum = small.tile([P, 1], f32, tag="ssum", name="ssum")
            nc.vector.tensor_add(ssum, sqs[0], sqs[1])
            mean = small.tile([P, 1], f32, tag="mean", name="mean")
            nc.vector.tensor_scalar_mul(mean, ysum, 1.0 / D)
            msq = small.tile([P, 1], f32, tag="msq", name="msq")
            nc.vector.tensor_mul(msq, mean, mean)
            var = small.tile([P, 1], f32, tag="var", name="var")
            nc.vector.scalar_tensor_tensor(
                out=var, in0=ssum, scalar=1.0 / D, in1=msq,
                op0=Alu.mult, op1=Alu.subtract,
            )
            std = small.tile([P, 1], f32, tag="std", name="std")
            nc.scalar.activation(out=std, in_=var, func=Act.Sqrt, bias=eps_sb)
            rstd = small.tile([P, 1], f32, tag="rstd", name="rstd")
            nc.vector.reciprocal(rstd, std)
            bact = small.tile([P, 1], f32, tag="bact", name="bact")
            nc.vector.scalar_tensor_tensor(
                out=bact, in0=mean, scalar=-1.0, in1=rstd,
                op0=Alu.mult, op1=Alu.mult,
            )
            out_sb = lnpool.tile([P, D], f32, tag="osb", name="out_sb")
            nc.scalar.activation(out=out_sb, in_=y_sb, func=Act.Identity,
                                 bias=bact, scale=rstd)
            if apply_affine:
                nc.vector.tensor_mul(out_sb, out_sb, g_bc)
                nc.vector.tensor_add(out_sb, out_sb, b_bc)
            nc.sync.dma_start(out=yo[cs, :], in_=out_sb)

    nc.compile()
    return nc


def _get_program(apply_affine: bool):
    key = ("prog", apply_affine)
    if key not in _CACHE:
        _CACHE[key] = _build_program(apply_affine)
    return _CACHE[key]


def _host_prep(inputs):
    x = np.asarray(inputs["x"], np.float32)
    xs = np.asarray(inputs["xs"], np.float32)
    w_qs = np.asarray(inputs["w_qs"], np.float32)
    b_qs = np.asarray(inputs["b_qs"], np.float32)
    w_ks = np.asarray(inputs["w_ks"], np.float32)
    w_vs = np.asarray(inputs["w_vs"], np.float32)
    b_vs = np.asarray(inputs["b_vs"], np.float32)
    w_fc = np.asarray(inputs["w_fc"], np.float32)
    b_fc = np.asarray(inputs["b_fc"], np.float32)
    ln_g = np.asarray(inputs["ln_g"], np.float32)
    ln_b = np.asarray(inputs["ln_b"], np.float32)

    apply_affine = not (np.all(ln_g == 1.0) and np.all(ln_b == 0.0))

    bprime = (b_vs @ w_fc + b_fc).astype(np.float32)

    mask = np.full((CL, P), NEG, np.float32)
    for t in range(CL):
        mask[t, t:t + 2 * NEI + 1] = 0.0   # window cols beyond CL+6 stay NEG
    mask2 = np.concatenate([mask, mask], axis=1)

    shared = {
        "wq": np.ascontiguousarray(w_qs.astype(BF16)),
        "wk": np.ascontiguousarray(w_ks.astype(BF16)),
        "wv": np.ascontiguousarray(w_vs.astype(BF16)),
        "wf": np.ascontiguousarray(w_fc.astype(BF16)),
        "bq": np.ascontiguousarray(b_qs.reshape(ND, P).T.astype(np.float32)),
        "bpr": np.ascontiguousarray(bprime.reshape(1, D).astype(BF16)),
        "msk": np.ascontiguousarray(mask2),
        "idn": np.eye(P, dtype=BF16),
        "ones": np.ones((1, P), BF16),
    }
    if apply_affine:
        shared["lng"] = np.ascontiguousarray(ln_g.reshape(1, D))
        shared["lnb"] = np.ascontiguousarray(ln_b.reshape(1, D))

    in_maps = []
    half_n = S // 2  # 1024
    for core in range(NCORES):
        b, half = core // 2, core % 2
        t0 = half * half_n
        xq = x[b, t0:t0 + half_n]
        halo = np.zeros((TH, D), np.float32)
        lo = max(0, t0 - NEI)
        hi = min(S, t0 + half_n + NEI)
        halo[lo - (t0 - NEI):hi - (t0 - NEI)] = xs[b, lo:hi]
        m = dict(shared)
        m["xq32"] = np.ascontiguousarray(xq)
        m["xqT"] = np.ascontiguousarray(xq.T.astype(BF16))
        m["xsT"] = np.ascontiguousarray(halo.T.astype(BF16))
        in_maps.append(m)
    return in_maps, apply_affine


def _run(inputs, trace=False, trace_kwargs=None):
    from concourse.bass_utils import run_bass_kernel_spmd

    in_maps, apply_affine = _host_prep(inputs)
    nc = _get_program(apply_affine)
    res = run_bass_kernel_spmd(
        nc, in_maps, list(range(NCORES)),
        trace=trace, **(trace_kwargs or {})
    )
    y = np.empty((B, S, D), np.float32)
    half_n = S // 2
    for core in range(NCORES):
        b, half = core // 2, core % 2
        y[b, half * half_n:(half + 1) * half_n] = res.results[core]["yo"]
    return y, res


def kernel(**inputs):
    y, _ = _run(inputs)
    return y



# revision 4
# speedup vs baseline: 1.1985x; 1.1985x over previous
"""LocalSelfAttention (window=7) Trainium2 Bass kernel — pipelined v2.

Full inputs in, full output out. Sharding: 8 cores = batch(4) x seq-half(2),
each core handles 1024 tokens with a 3-token zero-padded halo on xs.

Math notes (exact rewrites of the reference):
- reference projects zero-PADDED xs patches, so out-of-range taps have
  k = b_ks, v = b_vs. Softmax over taps is invariant to the per-(t,h)
  constant q . b_ks, so the K bias drops entirely; softmax weights sum to 1,
  so the V bias contributes exactly b_vs to o. Both b_vs @ w_fc and b_fc are
  folded into the residual on the host: xq = x + b_vs @ w_fc + b_fc.

Structure (per core):
- Q projection (dc-outer, 8 psum banks) -> QT feature-major bf16.
- K projection over the 1056-wide halo in 3 stripes -> KT feature-major.
- Software-pipelined chunk loop over 11 chunks of 96 tokens:
  PE issue order interleaves score matmuls (chunk ci), V projection
  (chunk ci+1), prob-transposes + PV (chunk ci, delayed 3 pair-slots),
  and FC matmuls (128-token chunks, dependency-mapped) so the tensor
  engine never sits behind the softmax chain.  Elementwise work is spread
  over DVE (mask+scale stt, reduces, recip, pt-copy, FC residual-add),
  ACT (exp, V/OT evictions, LN squares + final scale), and Pool/GpSimd
  (prob normalization, LN scalar chain) - Pool cannot touch PSUM.
"""

import sys

for _p in ("/opt/trn_rl_repo",):
    if _p not in sys.path:
        sys.path.insert(0, _p)

import numpy as np
import ml_dtypes

BF16 = ml_dtypes.bfloat16

H, DK, DV, D = 16, 64, 64, 1024
NEI = 3
TEMP = 8.0
EPS = 1e-5
B, S = 4, 2048
NCORES = 8
T = (B * S) // NCORES          # 1024 tokens per core
TH = T + 2 * NEI               # 1030 halo tokens
P = 128
NT = T // P                    # 8 fc-phase token chunks (128 tokens)
ND = D // P                    # 8 feature chunks
CL = 96                        # attention chunk length
NCH = 11
CST = [96 * i for i in range(10)] + [928]          # attn chunk starts
TH2 = 1056                     # padded halo width
NEG = -30000.0
KSTRIPES = [(0, 384), (384, 384), (768, 288)]      # K projection stripes
# FC chunk c (tokens 128c..128c+128) is emitted during attn chunk FCMAP^-1:
# dep(c) = first attn chunk index ci with 96ci+96 >= 128c+128; emit at dep+1.
FCMAP = {2: 0, 3: 1, 4: 2, 6: 3, 7: 4, 8: 5, 10: 6}   # ci -> fc chunk
OFF = 3                        # pair-slots between scores and transpose/PV

_CACHE = {}


def _build_program(apply_affine: bool):
    import concourse.bacc as bacc
    import concourse.tile as tile
    from concourse import mybir
    from contextlib import ExitStack

    f32 = mybir.dt.float32
    bf16 = mybir.dt.bfloat16
    Alu = mybir.AluOpType
    Act = mybir.ActivationFunctionType

    nc = bacc.Bacc(
        "TRN2", target_bir_lowering=False, debug=False, enable_asserts=False
    )

    def din(name, shape, dt_):
        return nc.dram_tensor(name, shape, dt_, kind="ExternalInput").ap()

    xq = din("xq", (T, D), bf16)         # residual + folded fc/v bias
    xqT = din("xqT", (D, T), bf16)       # x^T (host-transposed)
    xsT = din("xsT", (D, TH), bf16)      # xs^T with halo (host-transposed)
    wq = din("wq", (D, D), bf16)
    wk = din("wk", (D, D), bf16)
    wv = din("wv", (D, D), bf16)
    wf = din("wf", (D, D), bf16)
    bq = din("bq", (P, ND), f32)         # b_qs laid out [p, ec]
    msk = din("msk", (CL, 2 * P), f32)   # band mask 0 / NEG, two head slots
    idn = din("idn", (CL, CL), bf16)     # identity for PE transpose
    if apply_affine:
        lng = din("lng", (1, D), f32)
        lnb = din("lnb", (1, D), f32)
    yo = nc.dram_tensor("yo", (T, D), f32, kind="ExternalOutput").ap()

    with tile.TileContext(nc) as tc, ExitStack() as ctx:
        consts = ctx.enter_context(tc.tile_pool(name="consts", bufs=1))
        big = ctx.enter_context(tc.tile_pool(name="big", bufs=1))
        vpool = ctx.enter_context(tc.tile_pool(name="vpool", bufs=3))
        pmp = ctx.enter_context(tc.tile_pool(name="pmp", bufs=2))
        pep = ctx.enter_context(tc.tile_pool(name="pep", bufs=3))
        pnp = ctx.enter_context(tc.tile_pool(name="pnp", bufs=3))
        ptp = ctx.enter_context(tc.tile_pool(name="ptp", bufs=2))
        small = ctx.enter_context(tc.tile_pool(name="small", bufs=4))
        lnpool = ctx.enter_context(tc.tile_pool(name="lnpool", bufs=2))

        # ---- constants ----
        msk_sb = consts.tile([CL, 2 * P], f32, tag="msk")
        nc.sync.dma_start(out=msk_sb, in_=msk)
        idn_sb = consts.tile([CL, CL], bf16, tag="idn")
        nc.sync.dma_start(out=idn_sb, in_=idn)
        bq_sb = consts.tile([P, ND], f32, tag="bq")
        nc.sync.dma_start(out=bq_sb, in_=bq)
        eps_sb = consts.tile([P, 1], f32, tag="eps")
        nc.vector.memset(eps_sb, EPS)
        if apply_affine:
            import concourse.bass as bass

            g_bc = consts.tile([P, D], f32, tag="g_bc")
            b_bc = consts.tile([P, D], f32, tag="b_bc")
            nc.sync.dma_start(
                out=g_bc,
                in_=bass.AP(tensor=lng.tensor, offset=lng.offset,
                            ap=[[0, P]] + list(lng.ap[1:])),
            )
            nc.sync.dma_start(
                out=b_bc,
                in_=bass.AP(tensor=lnb.tensor, offset=lnb.offset,
                            ap=[[0, P]] + list(lnb.ap[1:])),
            )

        # ---- input loads: (wq,xT) pairs first so Q proj can start early ----
        wq_t, xT_t = [], []
        for dc in range(ND):
            w1 = big.tile([P, D], bf16, tag=f"wq{dc}", name=f"wq{dc}")
            nc.sync.dma_start(out=w1, in_=wq[dc * P:(dc + 1) * P, :])
            wq_t.append(w1)
            t1 = big.tile([P, T], bf16, tag=f"xT{dc}", name=f"xT{dc}")
            nc.sync.dma_start(out=t1, in_=xqT[dc * P:(dc + 1) * P, :])
            xT_t.append(t1)
        wk_t, xsT_t = [], []
        for dc in range(ND):
            w2 = big.tile([P, D], bf16, tag=f"wk{dc}", name=f"wk{dc}")
            nc.sync.dma_start(out=w2, in_=wk[dc * P:(dc + 1) * P, :])
            wk_t.append(w2)
            t2 = big.tile([P, TH2], bf16, tag=f"xsT{dc}", name=f"xsT{dc}")
            nc.sync.dma_start(out=t2[:, 0:TH], in_=xsT[dc * P:(dc + 1) * P, :])
            nc.vector.memset(t2[:, TH:TH2], 0.0)
            xsT_t.append(t2)
        wv_t, wf_t = [], []
        for dc in range(ND):
            w3 = big.tile([P, D], bf16, tag=f"wv{dc}", name=f"wv{dc}")
            nc.sync.dma_start(out=w3, in_=wv[dc * P:(dc + 1) * P, :])
            wv_t.append(w3)
        for dc in range(ND):
            w4 = big.tile([P, D], bf16, tag=f"wf{dc}", name=f"wf{dc}")
            nc.sync.dma_start(out=w4, in_=wf[dc * P:(dc + 1) * P, :])
            wf_t.append(w4)
        xr_t = []
        for c in range(NT):
            xr = big.tile([P, D], bf16, tag=f"xr{c}", name=f"xr{c}")
            nc.sync.dma_start(out=xr, in_=xq[c * P:(c + 1) * P, :])
            xr_t.append(xr)

        QT = [big.tile([P, T], bf16, tag=f"QT{ec}", name=f"QT{ec}")
              for ec in range(ND)]
        KT = [big.tile([P, TH2], bf16, tag=f"KT{ec}", name=f"KT{ec}")
              for ec in range(ND)]
        OT = [big.tile([P, T], bf16, tag=f"OT{ec}", name=f"OT{ec}")
              for ec in range(ND)]

        # ---- Q projection: dc-outer, 8 psum banks, ACT evict w/ bias ----
        with tc.tile_pool(name="psP", bufs=1, space="PSUM") as psP:
            for half in (0, 1):
                hs = slice(half * 512, (half + 1) * 512)
                ps = [psP.tile([P, 512], f32, tag=f"pp{ec}", name=f"pp{ec}")
                      for ec in range(ND)]
                for dc in range(ND):
                    for ec in range(ND):
                        nc.tensor.matmul(
                            ps[ec],
                            lhsT=wq_t[dc][:, ec * P:(ec + 1) * P],
                            rhs=xT_t[dc][:, hs],
                            start=(dc == 0), stop=(dc == ND - 1),
                        )
                for ec in range(ND):
                    nc.scalar.activation(out=QT[ec][:, hs], in_=ps[ec],
                                         func=Act.Identity,
                                         bias=bq_sb[:, ec:ec + 1], scale=1.0)
            # ---- K projection: 3 stripes over the 1056 halo, DVE evict ----
            for s0, w in KSTRIPES:
                ps = [psP.tile([P, 384], f32, tag=f"pp{ec}", name=f"pk{ec}")
                      for ec in range(ND)]
                for dc in range(ND):
                    for ec in range(ND):
                        nc.tensor.matmul(
                            ps[ec][:, 0:w],
                            lhsT=wk_t[dc][:, ec * P:(ec + 1) * P],
                            rhs=xsT_t[dc][:, s0:s0 + w],
                            start=(dc == 0), stop=(dc == ND - 1),
                        )
                for ec in range(ND):
                    nc.vector.tensor_copy(KT[ec][:, s0:s0 + w], ps[ec][:, 0:w])

        # ---- pipelined chunk loop ----
        with tc.tile_pool(name="psS", bufs=2, space="PSUM") as psS, \
             tc.tile_pool(name="psV", bufs=1, space="PSUM") as psV, \
             tc.tile_pool(name="psT", bufs=1, space="PSUM") as psT, \
             tc.tile_pool(name="psO", bufs=1, space="PSUM") as psO, \
             tc.tile_pool(name="psF", bufs=1, space="PSUM") as psF:

            V_tiles = {}

            def v_thunks(ci):
                """16 matmuls + 2 evictions projecting V for chunk ci."""
                s = CST[ci]
                vt = vpool.tile([P, D], bf16, tag="vt", name=f"V{ci}")
                V_tiles[ci] = vt
                out = []
                for half in (0, 1):
                    hs = slice(half * 512, (half + 1) * 512)
                    pv = [None]

                    def mk(dc, half=half, hs=hs, pv=pv, s=s, vt=vt):
                        def f():
                            if dc == 0:
                                pv[0] = psV.tile([P, 512], f32, tag="pv",
                                                 name="ps_v")
                            nc.tensor.matmul(
                                pv[0],
                                lhsT=xsT_t[dc][:, s:s + P],
                                rhs=wv_t[dc][:, hs],
                                start=(dc == 0), stop=(dc == ND - 1),
                            )
                            if dc == ND - 1:
                                nc.scalar.activation(out=vt[:, hs], in_=pv[0],
                                                     func=Act.Copy)
                        return f
                    out.extend(mk(dc) for dc in range(ND))
                return out

            def ln_block(c, y_sb):
                """LayerNorm for FC chunk c from y_sb; emits + output DMA."""
                cs = slice(c * P, (c + 1) * P)
                ysum = small.tile([P, 1], f32, tag="ysum", name="ysum")
                nc.vector.tensor_reduce(out=ysum, in_=y_sb,
                                        axis=mybir.AxisListType.X, op=Alu.add)
                sq0 = small.tile([P, 1], f32, tag="sq0", name="sq0")
                sq1 = small.tile([P, 1], f32, tag="sq1", name="sq1")
                ysq = lnpool.tile([P, 512], f32, tag="ysq", name="ysq")
                nc.scalar.activation(out=ysq, in_=y_sb[:, 0:512],
                                     func=Act.Square, accum_out=sq0)
                nc.scalar.activation(out=ysq, in_=y_sb[:, 512:1024],
                                     func=Act.Square, accum_out=sq1)
                ssum = small.tile([P, 1], f32, tag="ssum", name="ssum")
                nc.gpsimd.tensor_add(ssum, sq0, sq1)
                mean = small.tile([P, 1], f32, tag="mean", name="mean")
                nc.gpsimd.tensor_scalar_mul(mean, ysum, 1.0 / D)
                msq = small.tile([P, 1], f32, tag="msq", name="msq")
                nc.gpsimd.tensor_mul(msq, mean, mean)
                var = small.tile([P, 1], f32, tag="var", name="var")
                nc.vector.scalar_tensor_tensor(
                    out=var, in0=ssum, scalar=1.0 / D, in1=msq,
                    op0=Alu.mult, op1=Alu.subtract,
                )
                std = small.tile([P, 1], f32, tag="std", name="std")
                nc.scalar.activation(out=std, in_=var, func=Act.Sqrt,
                                     bias=eps_sb)
                rstd = small.tile([P, 1], f32, tag="rstd", name="rstd")
                nc.vector.reciprocal(rstd, std)
                bact = small.tile([P, 1], f32, tag="bact", name="bact")
                nc.vector.scalar_tensor_tensor(
                    out=bact, in0=mean, scalar=-1.0, in1=rstd,
                    op0=Alu.mult, op1=Alu.mult,
                )
                out_sb = lnpool.tile([P, D], f32, tag="osb", name="out_sb")
                nc.scalar.activation(out=out_sb, in_=y_sb, func=Act.Identity,
                                     bias=bact, scale=rstd)
                if apply_affine:
                    nc.vector.tensor_mul(out_sb, out_sb, g_bc)
                    nc.vector.tensor_add(out_sb, out_sb, b_bc)
                nc.sync.dma_start(out=yo[cs, :], in_=out_sb)

            def fc_thunks(c):
                """16 matmuls + 2 residual-stts + LN for 128-token chunk c."""
                cs = slice(c * P, (c + 1) * P)
                y_sb = lnpool.tile([P, D], f32, tag="ysb", name="y_sb")
                out = []
                for half in (0, 1):
                    hs = slice(half * 512, (half + 1) * 512)
                    pf = [None]

                    def mk(ec, half=half, hs=hs, pf=pf, cs=cs, y_sb=y_sb,
                           c=c):
                        def f():
                            if ec == 0:
                                pf[0] = psF.tile([P, 512], f32, tag="pf",
                                                 name="ps_f")
                            nc.tensor.matmul(
                                pf[0],
                                lhsT=OT[ec][:, cs],
                                rhs=wf_t[ec][:, hs],
                                start=(ec == 0), stop=(ec == ND - 1),
                            )
                            if ec == ND - 1:
                                nc.vector.scalar_tensor_tensor(
                                    out=y_sb[:, hs], in0=pf[0], scalar=1.0,
                                    in1=xr_t[c][:, hs],
                                    op0=Alu.mult, op1=Alu.add,
                                )
                        return f
                    out.extend(mk(ec) for ec in range(ND))
                out.append(lambda: ln_block(c, y_sb))
                return out

            # ---- attention pieces ----
            gstate = {}

            def emit_scores(ci, p):
                s = CST[ci]
                g = p // 2
                s2 = psS.tile([CL, 1024], f32, tag="s2", name="s2")
                nc.tensor.matmul(
                    s2[:, 0:P],
                    lhsT=QT[p][0:64, s:s + CL],
                    rhs=KT[p][0:64, s:s + P],
                    start=True, stop=True,
                )
                nc.tensor.matmul(
                    s2[:, 512:512 + P],
                    lhsT=QT[p][64:128, s:s + CL],
                    rhs=KT[p][64:128, s:s + P],
                    start=True, stop=True,
                )
                if p % 2 == 0:
                    pm = pmp.tile([CL, 512], f32, tag="pm", name="pm")
                    gstate[(ci, g)] = {"pm": pm}
                pm = gstate[(ci, g)]["pm"]
                half = p % 2
                nc.vector.scalar_tensor_tensor(
                    out=pm.rearrange("a (h w) -> a h w", h=4)[:, 2 * half:2 * half + 2, :],
                    in0=s2.rearrange("a (b w) -> a b w", b=2)[:, :, 0:P],
                    scalar=1.0 / TEMP,
                    in1=msk_sb.rearrange("a (h w) -> a h w", h=2),
                    op0=Alu.mult, op1=Alu.add,
                )

            def emit_softmax(ci, g):
                st = gstate[(ci, g)]
                pm = st["pm"]
                pe = pep.tile([CL, 512], bf16, tag="pe", name="pe")
                nc.scalar.activation(out=pe, in_=pm, func=Act.Exp)
                rs = small.tile([CL, 4], f32, tag="rs", name="rs")
                nc.vector.tensor_reduce(
                    out=rs, in_=pe.rearrange("a (h w) -> a h w", h=4),
                    axis=mybir.AxisListType.X, op=Alu.add,
                )
                rsr = small.tile([CL, 4], f32, tag="rsr", name="rsr")
                nc.vector.reciprocal(rsr, rs)
                pn = pnp.tile([CL, 512], bf16, tag="pn", name="pn")
                nc.gpsimd.tensor_tensor(
                    pn.rearrange("a (h w) -> a h w", h=4),
                    pe.rearrange("a (h w) -> a h w", h=4),
                    rsr[:, :, None].to_broadcast((CL, 4, P)),
                    Alu.mult,
                )
                st["pn"] = pn

            def emit_tpv(ci, g):
                st = gstate.pop((ci, g))
                pn = st["pn"]
                s = CST[ci]
                pt = psT.tile([P, 4 * CL], bf16, tag="pt", name="pt")
                pn4 = pn.rearrange("a (h w) -> a h w", h=4)
                for h in range(4):
                    nc.tensor.transpose(
                        pt[:, h * CL:(h + 1) * CL], pn4[:, h, :],
                        idn_sb,
                    )
                ptsb = ptp.tile([P, 4 * CL], bf16, tag="ptsb", name="ptsb")
                nc.vector.tensor_copy(ptsb, pt)
                vt = V_tiles[ci]
                for j in (0, 1):
                    pair = 2 * g + j
                    ot = psO.tile([P, CL], f32, tag="ot", name="ot")
                    nc.tensor.matmul(
                        ot[0:64, :],
                        lhsT=vt[:, pair * P:pair * P + 64],
                        rhs=ptsb[:, (2 * j) * CL:(2 * j + 1) * CL],
                        start=True, stop=True,
                    )
                    nc.tensor.matmul(
                        ot[64:128, :],
                        lhsT=vt[:, pair * P + 64:(pair + 1) * P],
                        rhs=ptsb[:, (2 * j + 1) * CL:(2 * j + 2) * CL],
                        start=True, stop=True,
                    )
                    nc.scalar.activation(out=OT[pair][:, s:s + CL], in_=ot,
                                         func=Act.Copy)

            # ---- the pipeline ----
            from collections import deque

            vq = deque()
            fq = deque()
            gfifo = deque()
            slot = 0

            for f in v_thunks(0):      # prologue: project V(0) densely
                f()

            for ci in range(NCH):
                if ci < NCH - 1:
                    vq.extend(v_thunks(ci + 1))
                if ci in FCMAP:
                    fq.extend(fc_thunks(FCMAP[ci]))
                for p in range(8):
                    emit_scores(ci, p)
                    if p % 2 == 1:
                        emit_softmax(ci, p // 2)
                        gfifo.append((ci, p // 2, slot))
                    for _ in range(3):
                        if vq:
                            vq.popleft()()
                    if gfifo and slot - gfifo[0][2] >= OFF:
                        gci, gg, _ = gfifo.popleft()
                        emit_tpv(gci, gg)
                    if p >= 2:
                        for _ in range(3):
                            if fq:
                                fq.popleft()()
                    slot += 1

            # ---- epilogue: drain pending groups, then final FC chunk ----
            while gfifo:
                gci, gg, _ = gfifo.popleft()
                emit_tpv(gci, gg)
            while fq:
                fq.popleft()()
            for f in fc_thunks(7):
                f()

    nc.compile()
    return nc


def _get_program(apply_affine: bool):
    key = ("prog", apply_affine)
    if key not in _CACHE:
        _CACHE[key] = _build_program(apply_affine)
    return _CACHE[key]


def _host_prep(inputs):
    x = np.asarray(inputs["x"], np.float32)
    xs = np.asarray(inputs["xs"], np.float32)
    w_qs = np.asarray(inputs["w_qs"], np.float32)
    b_qs = np.asarray(inputs["b_qs"], np.float32)
    w_ks = np.asarray(inputs["w_ks"], np.float32)
    w_vs = np.asarray(inputs["w_vs"], np.float32)
    b_vs = np.asarray(inputs["b_vs"], np.float32)
    w_fc = np.asarray(inputs["w_fc"], np.float32)
    b_fc = np.asarray(inputs["b_fc"], np.float32)
    ln_g = np.asarray(inputs["ln_g"], np.float32)
    ln_b = np.asarray(inputs["ln_b"], np.float32)

    apply_affine = not (np.all(ln_g == 1.0) and np.all(ln_b == 0.0))

    bprime = (b_vs @ w_fc + b_fc).astype(np.float32)

    mask = np.full((CL, P), NEG, np.float32)
    for t in range(CL):
        mask[t, t:t + 2 * NEI + 1] = 0.0
    mask2 = np.concatenate([mask, mask], axis=1)

    shared = {
        "wq": np.ascontiguousarray(w_qs.astype(BF16)),
        "wk": np.ascontiguousarray(w_ks.astype(BF16)),
        "wv": np.ascontiguousarray(w_vs.astype(BF16)),
        "wf": np.ascontiguousarray(w_fc.astype(BF16)),
        "bq": np.ascontiguousarray(b_qs.reshape(ND, P).T.astype(np.float32)),
        "msk": np.ascontiguousarray(mask2),
        "idn": np.eye(CL, dtype=BF16),
    }
    if apply_affine:
        shared["lng"] = np.ascontiguousarray(ln_g.reshape(1, D))
        shared["lnb"] = np.ascontiguousarray(ln_b.reshape(1, D))

    in_maps = []
    half_n = S // 2  # 1024
    for core in range(NCORES):
        b, half = core // 2, core % 2
        t0 = half * half_n
        xqc = x[b, t0:t0 + half_n] + bprime[None, :]
        halo = np.zeros((TH, D), np.float32)
        lo = max(0, t0 - NEI)
        hi = min(S, t0 + half_n + NEI)
        halo[lo - (t0 - NEI):hi - (t0 - NEI)] = xs[b, lo:hi]
        m = dict(shared)
        m["xq"] = np.ascontiguousarray(xqc.astype(BF16))
        m["xqT"] = np.ascontiguousarray(x[b, t0:t0 + half_n].T.astype(BF16))
        m["xsT"] = np.ascontiguousarray(halo.T.astype(BF16))
        in_maps.append(m)
    return in_maps, apply_affine


def _run(inputs, trace=False, trace_kwargs=None):
    from concourse.bass_utils import run_bass_kernel_spmd

    in_maps, apply_affine = _host_prep(inputs)
    nc = _get_program(apply_affine)
    res = run_bass_kernel_spmd(
        nc, in_maps, list(range(NCORES)),
        trace=trace, **(trace_kwargs or {})
    )
    y = np.empty((B, S, D), np.float32)
    half_n = S // 2
    for core in range(NCORES):
        b, half = core // 2, core % 2
        y[b, half * half_n:(half + 1) * half_n] = res.results[core]["yo"]
    return y, res


def kernel(**inputs):
    y, _ = _run(inputs)
    return y


# revision 12
# speedup vs baseline: 1.3023x; 1.0866x over previous
"""LocalSelfAttention (window=7) Trainium2 Bass kernel — pipelined v2.

Full inputs in, full output out. Sharding: 8 cores = batch(4) x seq-half(2),
each core handles 1024 tokens with a 3-token zero-padded halo on xs.

Math notes (exact rewrites of the reference):
- reference projects zero-PADDED xs patches, so out-of-range taps have
  k = b_ks, v = b_vs. Softmax over taps is invariant to the per-(t,h)
  constant q . b_ks, so the K bias drops entirely; softmax weights sum to 1,
  so the V bias contributes exactly b_vs to o. Both b_vs @ w_fc and b_fc are
  folded into the residual on the host: xq = x + b_vs @ w_fc + b_fc.

Structure (per core):
- Q projection (dc-outer, 8 psum banks) -> QT feature-major bf16.
- K projection over the 1056-wide halo in 3 stripes -> KT feature-major.
- Software-pipelined chunk loop over 11 chunks of 96 tokens:
  PE issue order interleaves score matmuls (chunk ci), V projection
  (chunk ci+1), prob-transposes + PV (chunk ci, delayed 3 pair-slots),
  and FC matmuls (128-token chunks, dependency-mapped) so the tensor
  engine never sits behind the softmax chain.  Elementwise work is spread
  over DVE (mask+scale stt, reduces, recip, pt-copy, FC residual-add),
  ACT (exp, V/OT evictions, LN squares + final scale), and Pool/GpSimd
  (prob normalization, LN scalar chain) - Pool cannot touch PSUM.
"""

import sys

for _p in ("/opt/trn_rl_repo",):
    if _p not in sys.path:
        sys.path.insert(0, _p)

import numpy as np
import ml_dtypes

BF16 = ml_dtypes.bfloat16

H, DK, DV, D = 16, 64, 64, 1024
NEI = 3
TEMP = 8.0
EPS = 1e-5
B, S = 4, 2048
NCORES = 8
T = (B * S) // NCORES          # 1024 tokens per core
TH = T + 2 * NEI               # 1030 halo tokens
P = 128
NT = T // P                    # 8 fc-phase token chunks (128 tokens)
ND = D // P                    # 8 feature chunks
CL = 96                        # attention chunk length
NCH = 11
CST = [96 * i for i in range(10)] + [928]          # attn chunk starts
TH2 = 1056                     # padded halo width
NEG = -30000.0
KSTRIPES = [(0, 384), (384, 384), (768, 288)]      # K projection stripes
# FC chunk c (tokens 128c..128c+128) is emitted during attn chunk FCMAP^-1:
# dep(c) = first attn chunk index ci with 96ci+96 >= 128c+128; emit at dep+1.
FCMAP = {2: 0, 3: 1, 4: 2, 6: 3, 7: 4, 8: 5, 10: 6}   # ci -> fc chunk
OFF = 4                        # pair-slots between scores and transpose/PV

_CACHE = {}


def _build_program(apply_affine: bool):
    import concourse.bacc as bacc
    import concourse.tile as tile
    from concourse import mybir
    from contextlib import ExitStack

    f32 = mybir.dt.float32
    bf16 = mybir.dt.bfloat16
    Alu = mybir.AluOpType
    Act = mybir.ActivationFunctionType

    nc = bacc.Bacc(
        "TRN2", target_bir_lowering=False, debug=False, enable_asserts=False
    )

    def din(name, shape, dt_):
        return nc.dram_tensor(name, shape, dt_, kind="ExternalInput").ap()

    xq = din("xq", (T, D), bf16)         # residual + folded fc/v bias
    xqT = din("xqT", (D, T), bf16)       # x^T (host-transposed)
    xsT = din("xsT", (D, TH), bf16)      # xs^T with halo (host-transposed)
    wq = din("wq", (D, D), bf16)
    wk = din("wk", (D, D), bf16)
    wv = din("wv", (D, D), bf16)
    wf = din("wf", (D, D), bf16)
    bq = din("bq", (P, ND), f32)         # b_qs laid out [p, ec]
    msk = din("msk", (CL, 2 * P), f32)   # band mask 0 / NEG, two head slots
    idn = din("idn", (CL, CL), bf16)     # identity for PE transpose
    if apply_affine:
        lng = din("lng", (1, D), f32)
        lnb = din("lnb", (1, D), f32)
    yo = nc.dram_tensor("yo", (T, D), f32, kind="ExternalOutput").ap()

    with tile.TileContext(nc) as tc, ExitStack() as ctx:
        consts = ctx.enter_context(tc.tile_pool(name="consts", bufs=1))
        big = ctx.enter_context(tc.tile_pool(name="big", bufs=1))
        vpool = ctx.enter_context(tc.tile_pool(name="vpool", bufs=3))
        pmp = ctx.enter_context(tc.tile_pool(name="pmp", bufs=2))
        pep = ctx.enter_context(tc.tile_pool(name="pep", bufs=3))
        pnp = ctx.enter_context(tc.tile_pool(name="pnp", bufs=3))
        ptp = ctx.enter_context(tc.tile_pool(name="ptp", bufs=2))
        small = ctx.enter_context(tc.tile_pool(name="small", bufs=4))
        lnpool = ctx.enter_context(tc.tile_pool(name="lnpool", bufs=2))

        # ---- constants ----
        msk_sb = consts.tile([CL, 2 * P], f32, tag="msk")
        nc.sync.dma_start(out=msk_sb, in_=msk)
        idn_sb = consts.tile([CL, CL], bf16, tag="idn")
        nc.sync.dma_start(out=idn_sb, in_=idn)
        bq_sb = consts.tile([P, ND], f32, tag="bq")
        nc.sync.dma_start(out=bq_sb, in_=bq)
        eps_sb = consts.tile([P, 1], f32, tag="eps")
        nc.vector.memset(eps_sb, EPS)
        if apply_affine:
            import concourse.bass as bass

            g_bc = consts.tile([P, D], f32, tag="g_bc")
            b_bc = consts.tile([P, D], f32, tag="b_bc")
            nc.sync.dma_start(
                out=g_bc,
                in_=bass.AP(tensor=lng.tensor, offset=lng.offset,
                            ap=[[0, P]] + list(lng.ap[1:])),
            )
            nc.sync.dma_start(
                out=b_bc,
                in_=bass.AP(tensor=lnb.tensor, offset=lnb.offset,
                            ap=[[0, P]] + list(lnb.ap[1:])),
            )

        # ---- input loads: (wq,xT) pairs first so Q proj can start early ----
        wq_t, xT_t = [], []
        for dc in range(ND):
            w1 = big.tile([P, D], bf16, tag=f"wq{dc}", name=f"wq{dc}")
            nc.sync.dma_start(out=w1, in_=wq[dc * P:(dc + 1) * P, :])
            wq_t.append(w1)
            t1 = big.tile([P, T], bf16, tag=f"xT{dc}", name=f"xT{dc}")
            nc.sync.dma_start(out=t1, in_=xqT[dc * P:(dc + 1) * P, :])
            xT_t.append(t1)
        wk_t, xsT_t = [], []
        for dc in range(ND):
            w2 = big.tile([P, D], bf16, tag=f"wk{dc}", name=f"wk{dc}")
            nc.sync.dma_start(out=w2, in_=wk[dc * P:(dc + 1) * P, :])
            wk_t.append(w2)
            t2 = big.tile([P, TH2], bf16, tag=f"xsT{dc}", name=f"xsT{dc}")
            nc.sync.dma_start(out=t2[:, 0:TH], in_=xsT[dc * P:(dc + 1) * P, :])
            nc.vector.memset(t2[:, TH:TH2], 0.0)
            xsT_t.append(t2)
        wv_t, wf_t = [], []
        for dc in range(ND):
            w3 = big.tile([P, D], bf16, tag=f"wv{dc}", name=f"wv{dc}")
            nc.sync.dma_start(out=w3, in_=wv[dc * P:(dc + 1) * P, :])
            wv_t.append(w3)
        for dc in range(ND):
            w4 = big.tile([P, D], bf16, tag=f"wf{dc}", name=f"wf{dc}")
            nc.sync.dma_start(out=w4, in_=wf[dc * P:(dc + 1) * P, :])
            wf_t.append(w4)
        xr_t = []
        for c in range(NT):
            xr = big.tile([P, D], bf16, tag=f"xr{c}", name=f"xr{c}")
            nc.sync.dma_start(out=xr, in_=xq[c * P:(c + 1) * P, :])
            xr_t.append(xr)

        QT = [big.tile([P, T], bf16, tag=f"QT{ec}", name=f"QT{ec}")
              for ec in range(ND)]
        KT = [big.tile([P, TH2], bf16, tag=f"KT{ec}", name=f"KT{ec}")
              for ec in range(ND)]
        OT_all = big.tile([P, ND * T], bf16, tag="OT_all", name="OT_all")
        OTr = OT_all.rearrange("a (p w) -> a p w", p=ND)

        # ---- Q projection: dc-outer, 8 psum banks, ACT evict w/ bias ----
        with tc.tile_pool(name="psP", bufs=1, space="PSUM") as psP:
            for half in (0, 1):
                hs = slice(half * 512, (half + 1) * 512)
                ps = [psP.tile([P, 512], f32, tag=f"pp{ec}", name=f"pp{ec}")
                      for ec in range(ND)]
                for dc in range(ND):
                    for ec in range(ND):
                        nc.tensor.matmul(
                            ps[ec],
                            lhsT=wq_t[dc][:, ec * P:(ec + 1) * P],
                            rhs=xT_t[dc][:, hs],
                            start=(dc == 0), stop=(dc == ND - 1),
                        )
                for ec in range(ND):
                    nc.scalar.activation(out=QT[ec][:, hs], in_=ps[ec],
                                         func=Act.Identity,
                                         bias=bq_sb[:, ec:ec + 1], scale=1.0)
            # ---- K projection: 3 stripes over the 1056 halo, DVE evict ----
            for s0, w in KSTRIPES:
                ps = [psP.tile([P, 384], f32, tag=f"pp{ec}", name=f"pk{ec}")
                      for ec in range(ND)]
                for dc in range(ND):
                    for ec in range(ND):
                        nc.tensor.matmul(
                            ps[ec][:, 0:w],
                            lhsT=wk_t[dc][:, ec * P:(ec + 1) * P],
                            rhs=xsT_t[dc][:, s0:s0 + w],
                            start=(dc == 0), stop=(dc == ND - 1),
                        )
                for ec in range(ND):
                    nc.vector.tensor_copy(KT[ec][:, s0:s0 + w], ps[ec][:, 0:w])

        # ---- pipelined chunk loop ----
        with tc.tile_pool(name="psS", bufs=2, space="PSUM") as psS, \
             tc.tile_pool(name="psV", bufs=1, space="PSUM") as psV, \
             tc.tile_pool(name="psT", bufs=1, space="PSUM") as psT, \
             tc.tile_pool(name="psO", bufs=1, space="PSUM") as psO, \
             tc.tile_pool(name="psF", bufs=1, space="PSUM") as psF:

            V_tiles = {}

            def v_thunks(ci):
                """16 matmuls + 2 evictions projecting V for chunk ci."""
                s = CST[ci]
                vt = vpool.tile([P, D], bf16, tag="vt", name=f"V{ci}")
                V_tiles[ci] = vt
                out = []
                for half in (0, 1):
                    hs = slice(half * 512, (half + 1) * 512)
                    pv = [None]

                    def mk(dc, half=half, hs=hs, pv=pv, s=s, vt=vt):
                        def f():
                            if dc == 0:
                                pv[0] = psV.tile([P, 512], f32, tag="pv",
                                                 name="ps_v")
                            nc.tensor.matmul(
                                pv[0],
                                lhsT=xsT_t[dc][:, s:s + P],
                                rhs=wv_t[dc][:, hs],
                                start=(dc == 0), stop=(dc == ND - 1),
                            )
                            if dc == ND - 1:
                                nc.scalar.activation(out=vt[:, hs], in_=pv[0],
                                                     func=Act.Copy)
                        return f
                    out.extend(mk(dc) for dc in range(ND))
                return out

            def ln_block(c, y_sb, ysums):
                """LayerNorm for FC chunk c from y_sb; emits + output DMA.

                rstd = exp(-0.5*ln(var+eps)) keeps the ACT engine inside one
                activation table (natural_log_exp_and_others: copy/identity/
                exp/ln/square) - a Sqrt would force a 1.3us table reload.
                """
                cs = slice(c * P, (c + 1) * P)
                ysum = small.tile([P, 1], f32, tag="ysum", name="ysum")
                nc.gpsimd.tensor_add(ysum, ysums[0], ysums[1])
                sq0 = small.tile([P, 1], f32, tag="sq0", name="sq0")
                sq1 = small.tile([P, 1], f32, tag="sq1", name="sq1")
                ysq = lnpool.tile([P, 512], f32, tag="ysq", name="ysq")
                nc.scalar.activation(out=ysq, in_=y_sb[:, 0:512],
                                     func=Act.Square, accum_out=sq0)
                nc.scalar.activation(out=ysq, in_=y_sb[:, 512:1024],
                                     func=Act.Square, accum_out=sq1)
                ssum = small.tile([P, 1], f32, tag="ssum", name="ssum")
                nc.gpsimd.tensor_add(ssum, sq0, sq1)
                mean = small.tile([P, 1], f32, tag="mean", name="mean")
                nc.gpsimd.tensor_scalar_mul(mean, ysum, 1.0 / D)
                msq = small.tile([P, 1], f32, tag="msq", name="msq")
                nc.gpsimd.tensor_mul(msq, mean, mean)
                var = small.tile([P, 1], f32, tag="var", name="var")
                nc.vector.scalar_tensor_tensor(
                    out=var, in0=ssum, scalar=1.0 / D, in1=msq,
                    op0=Alu.mult, op1=Alu.subtract,
                )
                lnv = small.tile([P, 1], f32, tag="lnv", name="lnv")
                nc.scalar.activation(out=lnv, in_=var, func=Act.Ln,
                                     bias=eps_sb)
                rstd = small.tile([P, 1], f32, tag="rstd", name="rstd")
                nc.scalar.activation(out=rstd, in_=lnv, func=Act.Exp,
                                     scale=-0.5)
                bact = small.tile([P, 1], f32, tag="bact", name="bact")
                nc.vector.scalar_tensor_tensor(
                    out=bact, in0=mean, scalar=-1.0, in1=rstd,
                    op0=Alu.mult, op1=Alu.mult,
                )
                out_sb = lnpool.tile([P, D], f32, tag="osb", name="out_sb")
                nc.scalar.activation(out=out_sb, in_=y_sb, func=Act.Identity,
                                     bias=bact, scale=rstd)
                if apply_affine:
                    nc.vector.tensor_mul(out_sb, out_sb, g_bc)
                    nc.vector.tensor_add(out_sb, out_sb, b_bc)
                nc.sync.dma_start(out=yo[cs, :], in_=out_sb)

            def fc_thunks(c):
                """16 matmuls + 2 residual-stts + LN for 128-token chunk c."""
                cs = slice(c * P, (c + 1) * P)
                y_sb = lnpool.tile([P, D], f32, tag="ysb", name="y_sb")
                ysums = [None, None]
                out = []
                for half in (0, 1):
                    hs = slice(half * 512, (half + 1) * 512)
                    pf = [None]

                    def mk(ec, half=half, hs=hs, pf=pf, cs=cs, y_sb=y_sb,
                           c=c):
                        def f():
                            if ec == 0:
                                pf[0] = psF.tile([P, 512], f32, tag="pf",
                                                 name="ps_f")
                            nc.tensor.matmul(
                                pf[0],
                                lhsT=OTr[:, ec, cs],
                                rhs=wf_t[ec][:, hs],
                                start=(ec == 0), stop=(ec == ND - 1),
                            )
                            if ec == ND - 1:
                                ysums[half] = small.tile(
                                    [P, 1], f32, tag=f"ysm{half}",
                                    name="ysm")
                                nc.vector.scalar_tensor_tensor(
                                    out=y_sb[:, hs], in0=pf[0], scalar=1.0,
                                    in1=xr_t[c][:, hs],
                                    op0=Alu.mult, op1=Alu.add,
                                    accum_out=ysums[half],
                                )
                        return f
                    out.extend(mk(ec) for ec in range(ND))
                out.append(lambda: ln_block(c, y_sb, ysums))
                return out

            # ---- attention pieces ----
            gstate = {}

            def emit_scores(ci, p):
                s = CST[ci]
                g = p // 2
                s2 = psS.tile([CL, 1024], f32, tag="s2", name="s2")
                nc.tensor.matmul(
                    s2[:, 0:P],
                    lhsT=QT[p][0:64, s:s + CL],
                    rhs=KT[p][0:64, s:s + P],
                    start=True, stop=True,
                )
                nc.tensor.matmul(
                    s2[:, 512:512 + P],
                    lhsT=QT[p][64:128, s:s + CL],
                    rhs=KT[p][64:128, s:s + P],
                    start=True, stop=True,
                )
                if p % 2 == 0:
                    pm = pmp.tile([CL, 512], f32, tag="pm", name="pm")
                    gstate[(ci, g)] = {"pm": pm}
                pm = gstate[(ci, g)]["pm"]
                half = p % 2
                nc.vector.scalar_tensor_tensor(
                    out=pm.rearrange("a (h w) -> a h w", h=4)[:, 2 * half:2 * half + 2, :],
                    in0=s2.rearrange("a (b w) -> a b w", b=2)[:, :, 0:P],
                    scalar=1.0 / TEMP,
                    in1=msk_sb.rearrange("a (h w) -> a h w", h=2),
                    op0=Alu.mult, op1=Alu.add,
                )

            def emit_softmax(ci, g):
                st = gstate[(ci, g)]
                pm = st["pm"]
                pe = pep.tile([CL, 512], bf16, tag="pe", name="pe")
                nc.scalar.activation(out=pe, in_=pm, func=Act.Exp)
                rs = small.tile([CL, 4], f32, tag="rs", name="rs")
                nc.vector.tensor_reduce(
                    out=rs, in_=pe.rearrange("a (h w) -> a h w", h=4),
                    axis=mybir.AxisListType.X, op=Alu.add,
                )
                rsr = small.tile([CL, 4], f32, tag="rsr", name="rsr")
                nc.vector.reciprocal(rsr, rs)
                pn = pnp.tile([CL, 512], bf16, tag="pn", name="pn")
                nc.gpsimd.tensor_tensor(
                    pn.rearrange("a (h w) -> a h w", h=4),
                    pe.rearrange("a (h w) -> a h w", h=4),
                    rsr[:, :, None].to_broadcast((CL, 4, P)),
                    Alu.mult,
                )
                st["pn"] = pn

            def emit_tpv(ci, g):
                st = gstate.pop((ci, g))
                pn = st["pn"]
                s = CST[ci]
                pt = psT.tile([P, 4 * CL], bf16, tag="pt", name="pt")
                pn4 = pn.rearrange("a (h w) -> a h w", h=4)
                for h in range(4):
                    nc.tensor.transpose(
                        pt[:, h * CL:(h + 1) * CL], pn4[:, h, :],
                        idn_sb,
                    )
                ptsb = ptp.tile([P, 4 * CL], bf16, tag="ptsb", name="ptsb")
                if g % 2 == 0:
                    nc.vector.tensor_copy(ptsb, pt)
                else:
                    nc.scalar.activation(out=ptsb, in_=pt, func=Act.Copy)
                vt = V_tiles[ci]
                ot = psO.tile([P, 2 * CL], f32, tag="ot", name="ot")
                for j in (0, 1):
                    pair = 2 * g + j
                    js = slice(j * CL, (j + 1) * CL)
                    nc.tensor.matmul(
                        ot[0:64, js],
                        lhsT=vt[:, pair * P:pair * P + 64],
                        rhs=ptsb[:, (2 * j) * CL:(2 * j + 1) * CL],
                        start=True, stop=True,
                    )
                    nc.tensor.matmul(
                        ot[64:128, js],
                        lhsT=vt[:, pair * P + 64:(pair + 1) * P],
                        rhs=ptsb[:, (2 * j + 1) * CL:(2 * j + 2) * CL],
                        start=True, stop=True,
                    )
                oview = OTr[:, 2 * g:2 * g + 2, s:s + CL]
                nc.scalar.activation(
                    out=oview,
                    in_=ot.rearrange("a (j w) -> a j w", j=2),
                    func=Act.Copy)

            # ---- the pipeline ----
            from collections import deque

            vq = deque()
            fq = deque()
            gfifo = deque()
            slot = 0

            for f in v_thunks(0):      # prologue: project V(0) densely
                f()

            for ci in range(NCH):
                if ci < NCH - 1:
                    vq.extend(v_thunks(ci + 1))
                if ci in FCMAP:
                    fq.extend(fc_thunks(FCMAP[ci]))
                for p in range(8):
                    emit_scores(ci, p)
                    if p % 2 == 1:
                        emit_softmax(ci, p // 2)
                        gfifo.append((ci, p // 2, slot))
                    for _ in range(3):
                        if vq:
                            vq.popleft()()
                    if gfifo and slot - gfifo[0][2] >= OFF:
                        gci, gg, _ = gfifo.popleft()
                        emit_tpv(gci, gg)
                    if p >= 2:
                        for _ in range(3):
                            if fq:
                                fq.popleft()()
                    slot += 1

            # ---- epilogue: drain pending groups, then final FC chunk ----
            while gfifo:
                gci, gg, _ = gfifo.popleft()
                emit_tpv(gci, gg)
            while fq:
                fq.popleft()()
            for f in fc_thunks(7):
                f()

    nc.compile()
    return nc


def _get_program(apply_affine: bool):
    key = ("prog", apply_affine)
    if key not in _CACHE:
        _CACHE[key] = _build_program(apply_affine)
    return _CACHE[key]


def _host_prep(inputs):
    x = np.asarray(inputs["x"], np.float32)
    xs = np.asarray(inputs["xs"], np.float32)
    w_qs = np.asarray(inputs["w_qs"], np.float32)
    b_qs = np.asarray(inputs["b_qs"], np.float32)
    w_ks = np.asarray(inputs["w_ks"], np.float32)
    w_vs = np.asarray(inputs["w_vs"], np.float32)
    b_vs = np.asarray(inputs["b_vs"], np.float32)
    w_fc = np.asarray(inputs["w_fc"], np.float32)
    b_fc = np.asarray(inputs["b_fc"], np.float32)
    ln_g = np.asarray(inputs["ln_g"], np.float32)
    ln_b = np.asarray(inputs["ln_b"], np.float32)

    apply_affine = not (np.all(ln_g == 1.0) and np.all(ln_b == 0.0))

    bprime = (b_vs @ w_fc + b_fc).astype(np.float32)

    mask = np.full((CL, P), NEG, np.float32)
    for t in range(CL):
        mask[t, t:t + 2 * NEI + 1] = 0.0
    mask2 = np.concatenate([mask, mask], axis=1)

    shared = {
        "wq": np.ascontiguousarray(w_qs.astype(BF16)),
        "wk": np.ascontiguousarray(w_ks.astype(BF16)),
        "wv": np.ascontiguousarray(w_vs.astype(BF16)),
        "wf": np.ascontiguousarray(w_fc.astype(BF16)),
        "bq": np.ascontiguousarray(b_qs.reshape(ND, P).T.astype(np.float32)),
        "msk": np.ascontiguousarray(mask2),
        "idn": np.eye(CL, dtype=BF16),
    }
    if apply_affine:
        shared["lng"] = np.ascontiguousarray(ln_g.reshape(1, D))
        shared["lnb"] = np.ascontiguousarray(ln_b.reshape(1, D))

    in_maps = []
    half_n = S // 2  # 1024
    for core in range(NCORES):
        b, half = core // 2, core % 2
        t0 = half * half_n
        xqc = x[b, t0:t0 + half_n] + bprime[None, :]
        halo = np.zeros((TH, D), np.float32)
        lo = max(0, t0 - NEI)
        hi = min(S, t0 + half_n + NEI)
        halo[lo - (t0 - NEI):hi - (t0 - NEI)] = xs[b, lo:hi]
        m = dict(shared)
        m["xq"] = np.ascontiguousarray(xqc.astype(BF16))
        m["xqT"] = np.ascontiguousarray(x[b, t0:t0 + half_n].T.astype(BF16))
        m["xsT"] = np.ascontiguousarray(halo.T.astype(BF16))
        in_maps.append(m)
    return in_maps, apply_affine


def _run(inputs, trace=False, trace_kwargs=None):
    from concourse.bass_utils import run_bass_kernel_spmd

    in_maps, apply_affine = _host_prep(inputs)
    nc = _get_program(apply_affine)
    res = run_bass_kernel_spmd(
        nc, in_maps, list(range(NCORES)),
        trace=trace, **(trace_kwargs or {})
    )
    y = np.empty((B, S, D), np.float32)
    half_n = S // 2
    for core in range(NCORES):
        b, half = core // 2, core % 2
        y[b, half * half_n:(half + 1) * half_n] = res.results[core]["yo"]
    return y, res


def kernel(**inputs):
    y, _ = _run(inputs)
    return y
